# revision 2
# baseline (speedup 1.0000x reference)
"""Axial transformer block on 8 NeuronCores via jax/pmap (axon PJRT).

Sharding (per spec hint): data-parallel over the non-attended spatial axis.
- H-attention: shard W across 8 cores (12 columns each); BN batch stats
  computed globally via psum across cores.
- all_gather, then W-attention: shard H across 8 cores.
- all_gather, then FFN: shard H bands (halo via zero-pad, matching SAME conv).
"""
import functools
import numpy as np
import jax
import jax.numpy as jnp
from jax import lax

EPS = 1e-5
DIM = 96
KS = 96
GROUPS = 8
BATCH = 4
NCORES = 8
SH = KS // NCORES  # 12: shard width along the non-attended axis

_REL_IDX = (np.arange(KS)[:, None] - np.arange(KS)[None, :] + KS - 1).reshape(-1)


def _ln(x, w, b):
    m = x.mean(1, keepdims=True)
    v = ((x - m) ** 2).mean(1, keepdims=True)
    return (x - m) / jnp.sqrt(v + EPS) * w[None, :, None, None] + b[None, :, None, None]


def _bn_p(x, g, b, ax):
    """BatchNorm with batch statistics over ALL shards (psum across cores)."""
    axes = tuple(i for i in range(x.ndim) if i != ax)
    m = lax.pmean(x.mean(axes, keepdims=True), "i")
    e2 = lax.pmean((x * x).mean(axes, keepdims=True), "i")
    v = e2 - m * m
    sh = [1] * x.ndim
    sh[ax] = -1
    return (x - m) / jnp.sqrt(v + EPS) * g.reshape(sh) + b.reshape(sh)


def _softmax(x):
    m = x.max(axis=3, keepdims=True)
    e = jnp.exp(x - m)
    return e / e.sum(axis=3, keepdims=True)


def _axial_p(xb, wqkv, gq, bq, rel, gs, bs, go, bo, width, n, wd):
    # xb: (n*wd, C, L) rows of this shard
    gp = DIM // GROUPS
    qkv = _bn_p(jnp.einsum("oc,bcl->bol", wqkv, xb), gq, bq, 1)
    qkv = qkv.reshape(n * wd, GROUPS, 2 * gp, KS)
    q = qkv[:, :, : gp // 2]
    k = qkv[:, :, gp // 2 : gp]
    v = qkv[:, :, gp:]
    emb = rel[:, _REL_IDX].reshape(2 * gp, KS, KS)
    qe, ke, ve = emb[: gp // 2], emb[gp // 2 : gp], emb[gp:]
    qk = jnp.einsum("bgci,bgcj->bgij", q, k)
    qr = jnp.einsum("bgci,cij->bgij", q, qe)
    kr = jnp.einsum("bgci,cij->bgij", k, ke).transpose(0, 1, 3, 2)
    sim = _bn_p(jnp.concatenate([qk, qr, kr], 1), gs, bs, 1)
    sim = sim.reshape(n * wd, 3, GROUPS, KS, KS).sum(1)
    sim = _softmax(sim)
    sv = jnp.einsum("bgij,bgcj->bgci", sim, v)
    sve = jnp.einsum("bgij,cij->bgci", sim, ve)
    out = _bn_p(
        jnp.concatenate([sv, sve], -1).reshape(n * wd, 2 * DIM, KS), go, bo, 1
    )
    out = out.reshape(n, wd, DIM, 2, KS).sum(3)
    # width=False: rows are (n, w), L = H -> (N, C, H, wd)
    # width=True:  rows are (n, h), L = W -> (N, C, wd, W)
    return out.transpose(0, 2, 1, 3) if width else out.transpose(0, 2, 3, 1)


def _block(idx_arr, x, ln1_w, ln1_b,
           h_wqkv, h_gqkv, h_bqkv, h_rel, h_gsim, h_bsim, h_gout, h_bout,
           w_wqkv, w_gqkv, w_bqkv, w_rel, w_gsim, w_bsim, w_gout, w_bout,
           ln2_w, ln2_b, ffn_win, ffn_wdw, ffn_wout):
    idx = idx_arr[0]

    # ---- height attention, sharded over W ----
    xs = lax.dynamic_slice(x, (0, 0, 0, idx * SH), (BATCH, DIM, KS, SH))
    ys = _ln(xs, ln1_w, ln1_b)
    xb = ys.transpose(0, 3, 1, 2).reshape(BATCH * SH, DIM, KS)
    y1 = _axial_p(xb, h_wqkv, h_gqkv, h_bqkv, h_rel, h_gsim, h_bsim,
                  h_gout, h_bout, False, BATCH, SH)  # (N, C, H, SH)
    g = lax.all_gather(y1, "i")  # (8, N, C, H, SH)
    y1f = g.transpose(1, 2, 3, 0, 4).reshape(BATCH, DIM, KS, KS)

    # ---- width attention, sharded over H ----
    ys2 = lax.dynamic_slice(y1f, (0, 0, idx * SH, 0), (BATCH, DIM, SH, KS))
    xb2 = ys2.transpose(0, 2, 1, 3).reshape(BATCH * SH, DIM, KS)
    y2 = _axial_p(xb2, w_wqkv, w_gqkv, w_bqkv, w_rel, w_gsim, w_bsim,
                  w_gout, w_bout, True, BATCH, SH)  # (N, C, SH, W)
    g2 = lax.all_gather(y2, "i")  # (8, N, C, SH, W)
    y2f = g2.transpose(1, 2, 0, 3, 4).reshape(BATCH, DIM, KS, KS)

    # ---- residual + FFN, sharded over H bands with halo ----
    z = x + y2f
    zln = _ln(z, ln2_w, ln2_b)
    zp = jnp.pad(zln, ((0, 0), (0, 0), (1, 1), (0, 0)))
    band = lax.dynamic_slice(zp, (0, 0, idx * SH, 0), (BATCH, DIM, SH + 2, KS))
    h = jnp.einsum("oc,bchw->bohw", ffn_win, band)  # (N, 510, SH+2, W)
    hp = jnp.pad(h, ((0, 0), (0, 0), (0, 0), (1, 1)))
    w = ffn_wdw[:, 0]  # (510, 3, 3)
    conv = sum(
        w[None, :, dy, dx, None, None] * hp[:, :, dy : dy + SH, dx : dx + KS]
        for dy in range(3) for dx in range(3)
    )  # (N, 510, SH, W)
    x1, x2 = jnp.split(conv, 2, axis=1)
    gelu = 0.5 * x1 * (1.0 + lax.erf(x1 * np.float32(1.0 / np.sqrt(2.0))))
    ffn = jnp.einsum("oc,bchw->bohw", ffn_wout, gelu * x2)  # (N, 96, SH, W)
    zb = lax.dynamic_slice(z, (0, 0, idx * SH, 0), (BATCH, DIM, SH, KS))
    return zb + ffn  # (N, C, SH, W)


_PFN = None


def _get_pfn():
    global _PFN
    if _PFN is None:
        devs = jax.devices()[:NCORES]
        _PFN = jax.pmap(_block, axis_name="i", in_axes=(0,) + (None,) * 24,
                        devices=devs)
    return _PFN


def kernel(**inputs):
    order = ["x", "ln1_w", "ln1_b",
             "h_wqkv", "h_gqkv", "h_bqkv", "h_rel", "h_gsim", "h_bsim",
             "h_gout", "h_bout",
             "w_wqkv", "w_gqkv", "w_bqkv", "w_rel", "w_gsim", "w_bsim",
             "w_gout", "w_bout",
             "ln2_w", "ln2_b", "ffn_win", "ffn_wdw", "ffn_wout"]
    args = [np.asarray(inputs[k], dtype=np.float32) for k in order]
    idx_arr = np.arange(NCORES, dtype=np.int32).reshape(NCORES, 1)
    out = _get_pfn()(idx_arr, *args)  # (8, N, C, SH, W)
    out = np.asarray(out)
    out = out.transpose(1, 2, 0, 3, 4).reshape(BATCH, DIM, KS, KS)
    return out.astype(np.float32)


if __name__ == "__main__":
    rng = np.random.default_rng(0)
    ins = {"x": rng.standard_normal((BATCH, DIM, KS, KS), dtype=np.float32)}
    print(kernel(**ins).shape)


# revision 3
# speedup vs baseline: 9.9492x; 9.9492x over previous
"""Axial transformer block on 8 NeuronCores via jax/pmap (axon PJRT).

Sharding (per spec hint): data-parallel over the non-attended spatial axis.
- H-attention: shard W across 8 cores (12 columns each); BN batch stats
  computed globally via psum across cores.
- all_gather, then W-attention: shard H across 8 cores.
- all_gather, then FFN: shard H bands (halo via zero-pad, matching SAME conv).
"""
import functools
import numpy as np
import jax
import jax.numpy as jnp
from jax import lax

EPS = 1e-5
DIM = 96
KS = 96
GROUPS = 8
BATCH = 4
NCORES = 8
SH = KS // NCORES  # 12: shard width along the non-attended axis

_REL_IDX = (np.arange(KS)[:, None] - np.arange(KS)[None, :] + KS - 1).reshape(-1)


def _ln(x, w, b):
    m = x.mean(1, keepdims=True)
    v = ((x - m) ** 2).mean(1, keepdims=True)
    return (x - m) / jnp.sqrt(v + EPS) * w[None, :, None, None] + b[None, :, None, None]


def _bn_p(x, g, b, ax):
    """BatchNorm with batch statistics over ALL shards (psum across cores)."""
    axes = tuple(i for i in range(x.ndim) if i != ax)
    m = lax.pmean(x.mean(axes, keepdims=True), "i")
    e2 = lax.pmean((x * x).mean(axes, keepdims=True), "i")
    v = e2 - m * m
    sh = [1] * x.ndim
    sh[ax] = -1
    return (x - m) / jnp.sqrt(v + EPS) * g.reshape(sh) + b.reshape(sh)


def _softmax(x):
    m = x.max(axis=3, keepdims=True)
    e = jnp.exp(x - m)
    return e / e.sum(axis=3, keepdims=True)


def _axial_p(xb, wqkv, gq, bq, rel, gs, bs, go, bo, width, n, wd):
    # xb: (n*wd, C, L) rows of this shard
    gp = DIM // GROUPS
    qkv = _bn_p(jnp.einsum("oc,bcl->bol", wqkv, xb), gq, bq, 1)
    qkv = qkv.reshape(n * wd, GROUPS, 2 * gp, KS)
    q = qkv[:, :, : gp // 2]
    k = qkv[:, :, gp // 2 : gp]
    v = qkv[:, :, gp:]
    emb = rel[:, _REL_IDX].reshape(2 * gp, KS, KS)
    qe, ke, ve = emb[: gp // 2], emb[gp // 2 : gp], emb[gp:]
    qk = jnp.einsum("bgci,bgcj->bgij", q, k)
    qr = jnp.einsum("bgci,cij->bgij", q, qe)
    kr = jnp.einsum("bgci,cij->bgij", k, ke).transpose(0, 1, 3, 2)
    sim = _bn_p(jnp.concatenate([qk, qr, kr], 1), gs, bs, 1)
    sim = sim.reshape(n * wd, 3, GROUPS, KS, KS).sum(1)
    sim = _softmax(sim)
    sv = jnp.einsum("bgij,bgcj->bgci", sim, v)
    sve = jnp.einsum("bgij,cij->bgci", sim, ve)
    out = _bn_p(
        jnp.concatenate([sv, sve], -1).reshape(n * wd, 2 * DIM, KS), go, bo, 1
    )
    out = out.reshape(n, wd, DIM, 2, KS).sum(3)
    # width=False: rows are (n, w), L = H -> (N, C, H, wd)
    # width=True:  rows are (n, h), L = W -> (N, C, wd, W)
    return out.transpose(0, 2, 1, 3) if width else out.transpose(0, 2, 3, 1)


def _block(idx_arr, xw, xh, ln1_w, ln1_b,
           h_wqkv, h_gqkv, h_bqkv, h_rel, h_gsim, h_bsim, h_gout, h_bout,
           w_wqkv, w_gqkv, w_bqkv, w_rel, w_gsim, w_bsim, w_gout, w_bout,
           ln2_w, ln2_b, ffn_win, ffn_wdw, ffn_wout):
    idx = idx_arr[0]

    # ---- height attention, sharded over W ----
    # xw: (N, C, H, SH) this device's W-shard of x
    ys = _ln(xw, ln1_w, ln1_b)
    xb = ys.transpose(0, 3, 1, 2).reshape(BATCH * SH, DIM, KS)
    y1 = _axial_p(xb, h_wqkv, h_gqkv, h_bqkv, h_rel, h_gsim, h_bsim,
                  h_gout, h_bout, False, BATCH, SH)  # (N, C, H, SH)
    g = lax.all_gather(y1, "i")  # (8, N, C, H, SH)
    y1f = g.transpose(1, 2, 3, 0, 4).reshape(BATCH, DIM, KS, KS)

    # ---- width attention, sharded over H ----
    ys2 = lax.dynamic_slice(y1f, (0, 0, idx * SH, 0), (BATCH, DIM, SH, KS))
    xb2 = ys2.transpose(0, 2, 1, 3).reshape(BATCH * SH, DIM, KS)
    y2 = _axial_p(xb2, w_wqkv, w_gqkv, w_bqkv, w_rel, w_gsim, w_bsim,
                  w_gout, w_bout, True, BATCH, SH)  # (N, C, SH, W)
    g2 = lax.all_gather(y2, "i")  # (8, N, C, SH, W)
    y2f = g2.transpose(1, 2, 0, 3, 4).reshape(BATCH, DIM, KS, KS)

    # ---- residual + FFN, sharded over H bands with halo ----
    # xh: (N, C, SH+2, W) this device's H band of x with 1-row halo
    # (rows idx*SH-1 .. idx*SH+SH, zero-padded at the global edges)
    y2p = jnp.pad(y2f, ((0, 0), (0, 0), (1, 1), (0, 0)))
    yband = lax.dynamic_slice(y2p, (0, 0, idx * SH, 0),
                              (BATCH, DIM, SH + 2, KS))
    zband = xh + yband  # z on the halo band (halo rows of z, 0 at edges)
    # LN2 on the band: interior rows match the full-tensor LN (per-pixel);
    # edge halo rows are zeros after LN*0... careful: LN of a zero row is
    # 0/sqrt(var+eps)*w + b = b. The reference pads AFTER LN2 with zeros,
    # so zero the halo rows explicitly when they are global edges.
    zln = _ln(zband, ln2_w, ln2_b)
    row = jnp.arange(SH + 2)[None, None, :, None] + idx * SH - 1
    valid = (row >= 0) & (row < KS)
    zln = jnp.where(valid, zln, 0.0)
    h = jnp.einsum("oc,bchw->bohw", ffn_win, zln)  # (N, 510, SH+2, W)
    hp = jnp.pad(h, ((0, 0), (0, 0), (0, 0), (1, 1)))
    w = ffn_wdw[:, 0]  # (510, 3, 3)
    conv = sum(
        w[None, :, dy, dx, None, None] * hp[:, :, dy : dy + SH, dx : dx + KS]
        for dy in range(3) for dx in range(3)
    )  # (N, 510, SH, W)
    x1, x2 = jnp.split(conv, 2, axis=1)
    gelu = 0.5 * x1 * (1.0 + lax.erf(x1 * np.float32(1.0 / np.sqrt(2.0))))
    ffn = jnp.einsum("oc,bchw->bohw", ffn_wout, gelu * x2)  # (N, 96, SH, W)
    zb = zband[:, :, 1:-1, :]
    return zb + ffn  # (N, C, SH, W)


def shard_x(x):
    """Per-device input shards: W-shards for H-attention and H-halo bands
    for the FFN/residual stage."""
    xw = np.stack([x[:, :, :, j * SH:(j + 1) * SH] for j in range(NCORES)])
    xp = np.pad(x, ((0, 0), (0, 0), (1, 1), (0, 0)))
    xh = np.stack([xp[:, :, j * SH:j * SH + SH + 2, :] for j in range(NCORES)])
    return xw, xh


_PFN = None


def _get_pfn():
    global _PFN
    if _PFN is None:
        devs = jax.devices()[:NCORES]
        _PFN = jax.pmap(_block, axis_name="i",
                        in_axes=(0, 0, 0) + (None,) * 23, devices=devs)
    return _PFN


def kernel(**inputs):
    order = ["x", "ln1_w", "ln1_b",
             "h_wqkv", "h_gqkv", "h_bqkv", "h_rel", "h_gsim", "h_bsim",
             "h_gout", "h_bout",
             "w_wqkv", "w_gqkv", "w_bqkv", "w_rel", "w_gsim", "w_bsim",
             "w_gout", "w_bout",
             "ln2_w", "ln2_b", "ffn_win", "ffn_wdw", "ffn_wout"]
    args = [np.asarray(inputs[k], dtype=np.float32) for k in order]
    x = args[0]
    xw, xh = shard_x(x)
    idx_arr = np.arange(NCORES, dtype=np.int32).reshape(NCORES, 1)
    out = _get_pfn()(idx_arr, xw, xh, *args[1:])  # (8, N, C, SH, W)
    out = np.asarray(out)
    out = out.transpose(1, 2, 0, 3, 4).reshape(BATCH, DIM, KS, KS)
    return out.astype(np.float32)


def _warmup():
    try:
        hid = int(DIM * 2.66)
        z = np.zeros
        ins = {"x": z((BATCH, DIM, KS, KS), np.float32),
               "ln1_w": z(DIM, np.float32), "ln1_b": z(DIM, np.float32),
               "ln2_w": z(DIM, np.float32), "ln2_b": z(DIM, np.float32),
               "ffn_win": z((2 * hid, DIM), np.float32),
               "ffn_wdw": z((2 * hid, 1, 3, 3), np.float32),
               "ffn_wout": z((DIM, hid), np.float32)}
        for p in ("h_", "w_"):
            ins[p + "wqkv"] = z((2 * DIM, DIM), np.float32)
            ins[p + "gqkv"] = z(2 * DIM, np.float32)
            ins[p + "bqkv"] = z(2 * DIM, np.float32)
            ins[p + "rel"] = z((2 * (DIM // GROUPS), 2 * KS - 1), np.float32)
            ins[p + "gsim"] = z(3 * GROUPS, np.float32)
            ins[p + "bsim"] = z(3 * GROUPS, np.float32)
            ins[p + "gout"] = z(2 * DIM, np.float32)
            ins[p + "bout"] = z(2 * DIM, np.float32)
        kernel(**ins)
    except Exception:
        pass


_warmup()

if __name__ == "__main__":
    rng = np.random.default_rng(0)
    ins = {"x": rng.standard_normal((BATCH, DIM, KS, KS), dtype=np.float32)}
    print(kernel(**ins).shape)
